# revision 1
# baseline (speedup 1.0000x reference)
"""Multi-head attention (B=4, L=2048, E=1024, H=16, DK=64) on 8 TRN2 cores.

Sharding: core c -> (batch b = c//2, head-group g = c%2 of 8 heads).
Per core: qkv projection for its batch+heads (bf16 matmuls, fp32 accum),
attention for 8 (b,h) pairs, partial fc over its 512 features, then a
pair-wise ReduceScatter so each core emits a disjoint 1024-token slice
of the final output. Host assembles the full [4, 2048, 1024] output.

Self-contained: hardcodes all shapes; requires only the concourse stack.
"""

import numpy as np
import ml_dtypes

try:
    import axon_prof

    axon_prof.install()
except Exception:
    pass

import concourse.mybir as mybir
import concourse.tile as tile
from concourse import bacc
from concourse import bass_utils

B, L, E = 4, 2048, 1024
H, DK = 16, 64
H8 = 8                      # heads per core
F = H8 * 3 * DK             # qkv features per core = 1536
FO = H8 * DK                # attn-out features per core = 512
NCORES = 8
LHALF = L // 2

f32 = mybir.dt.float32
bf16 = mybir.dt.bfloat16
Exp = mybir.ActivationFunctionType.Exp
MUL = mybir.AluOpType.mult
ADD = mybir.AluOpType.add

_CACHE = {}


def build_nc():
    nc = bacc.Bacc("TRN2", target_bir_lowering=False, debug=False, num_devices=NCORES)

    # x arrives already transposed (host-side) so no xbar transpose is needed
    x = nc.dram_tensor("x", [E, L], bf16, kind="ExternalInput")
    w_qkv = nc.dram_tensor("w_qkv", [E, F], bf16, kind="ExternalInput")
    b_qkv = nc.dram_tensor("b_qkv", [128, 12], f32, kind="ExternalInput")
    w_fc = nc.dram_tensor("w_fc", [FO, E], bf16, kind="ExternalInput")
    b_fc = nc.dram_tensor("b_fc", [1, E], f32, kind="ExternalInput")
    out = nc.dram_tensor("out", [LHALF, E], f32, kind="ExternalOutput")

    with tile.TileContext(nc) as tc:
        with (
            tc.tile_pool(name="persist", bufs=1) as pp,
            tc.tile_pool(name="work", bufs=3) as wp,
            tc.tile_pool(name="stage", bufs=1) as sp,
            tc.tile_pool(name="ys", bufs=3) as yp_pool,
            tc.tile_pool(name="dram", bufs=1, space="DRAM") as dram,
        ):
            # ---- persistent SBUF ----
            xT = pp.tile([128, 8, L], bf16, tag="xT")          # X^T  4 MiB
            wq = pp.tile([128, 8, F], bf16, tag="wq")          # 3 MiB
            bq = pp.tile([128, 12], f32, tag="bq")
            wfc = pp.tile([128, 4, E], bf16, tag="wfc")        # 1 MiB
            bias = pp.tile([128, E], f32, tag="bias")          # 0.5 MiB
            qt = pp.tile([128, 4, L], bf16, tag="qt")          # Q^T 2 MiB
            kt = pp.tile([128, 4, L], bf16, tag="kt")          # K^T 2 MiB
            vt = sp.tile([128, 4, L], bf16, tag="vt")          # V^T staging 2 MiB
            # V natural layout, 80-elem stride; col 64 holds the ones column
            # so AV matmuls with lhsT [V|1] (M=65) produce rowsums for free
            v = pp.tile([128, H8, 16, 80], bf16, tag="v")      # 2.5 MiB
            onT = pp.tile([128, 4, L], bf16, tag="onT")        # attn out^T 2 MiB

            # ---- input DMAs (plain copies, split across both HWDGE queues)
            for e in range(8):
                nc.scalar.dma_start(
                    wq[:, e, :], w_qkv[e * 128 : (e + 1) * 128, :]
                )
                nc.sync.dma_start(xT[:, e, :], x[e * 128 : (e + 1) * 128, :])
                if e == 0:
                    nc.scalar.dma_start(bq[:], b_qkv[:])
            nc.sync.dma_start(wfc[:], w_fc.rearrange("(c p) e -> p c e", p=128))
            bfc_row = pp.tile([1, E], f32, tag="bfc_row")
            nc.sync.dma_start(bfc_row[:], b_fc[:])
            nc.gpsimd.partition_broadcast(bias[:], bfc_row[:])
            nc.vector.memset(v[:, :, :, 64:65], 1.0)

            # ---- phase 1: qkv^T = W_shard.T @ X^T (+bias), bf16 ----
            # V tiles first so the V^T -> V xbar transposes hide under the
            # remaining Q/K matmuls instead of delaying the first exp.
            with tc.tile_pool(name="psq", bufs=2, space="PSUM") as psq_pool:
                for ft in (8, 9, 10, 11, 0, 4, 1, 5, 2, 6, 3, 7):
                    ps = psq_pool.tile([128, L], f32, tag="psq")
                    for kc in range(8):
                        lhsT = wq[:, kc, ft * 128 : (ft + 1) * 128]
                        for tb in range(4):
                            nc.tensor.matmul(
                                ps[:, tb * 512 : (tb + 1) * 512],
                                lhsT,
                                xT[:, kc, tb * 512 : (tb + 1) * 512],
                                start=(kc == 0),
                                stop=(kc == 7),
                            )
                    if ft < 4:
                        dst = qt[:, ft, :]
                    elif ft < 8:
                        dst = kt[:, ft - 4, :]
                    else:
                        dst = vt[:, ft - 8, :]
                    nc.vector.tensor_scalar_add(dst, ps[:], bq[:, ft : ft + 1])
                    if ft >= 8:
                        # V^T -> V (token-major) via xbar transpose, per head
                        for h in (2 * (ft - 8), 2 * (ft - 8) + 1):
                            nc.sync.dma_start_transpose(
                                v[:, h, :, 0:DK],
                                vt[(h % 2) * 64 : (h % 2) * 64 + 64, h // 2, :],
                            )

            # ---- phase 2+3: attention halves, each followed by partial fc
            # and a pair ReduceScatter (RS#1 overlaps the qb=1 half) ----
            # Attention is software-pipelined: AV(kk-1) is emitted after
            # S^T(kk)/exp(kk) so the PE computes AV while ACT runs exp.
            rs_in = [
                dram.tile([LHALF, E], bf16, name=f"rs_in{i}", tag=f"rs_in{i}")
                for i in range(2)
            ]
            rs_out1 = dram.tile([LHALF // 2, E], bf16, name="rs_out1", tag="rs_out1")
            rs_out2 = [
                dram.tile([LHALF // 2, E], bf16, name=f"rs_out2{i}", tag=f"rs_out2{i}")
                for i in range(1)
            ]
            PAIRS = [[0, 1], [2, 3], [4, 5], [6, 7]]
            with (
                tc.tile_pool(name="pst", bufs=1, space="PSUM") as pst_pool,
                tc.tile_pool(name="psav0", bufs=1, space="PSUM") as psav0_pool,
                tc.tile_pool(name="psav1", bufs=1, space="PSUM") as psav1_pool,
            ):

                def attn_block(qb, j):
                    av0 = psav0_pool.tile([128, LHALF], f32, tag="av0")
                    av1 = psav1_pool.tile([128, LHALF], f32, tag="av1")
                    pts = {}

                    def emit_st(kk):
                        st = pst_pool.tile([128, L], f32, tag="st", name="st")
                        for u in range(2):
                            q0 = qb * LHALF + u * 512
                            nc.tensor.matmul(
                                st[:, u * 512 : (u + 1) * 512],
                                kt[0:64, j, kk * 128 : (kk + 1) * 128],
                                qt[0:64, j, q0 : q0 + 512],
                                start=True,
                                stop=True,
                            )
                            nc.tensor.matmul(
                                st[:, 1024 + u * 512 : 1024 + (u + 1) * 512],
                                kt[64:128, j, kk * 128 : (kk + 1) * 128],
                                qt[64:128, j, q0 : q0 + 512],
                                start=True,
                                stop=True,
                            )
                        pt = wp.tile([128, L], bf16, tag="pt")
                        nc.scalar.activation(pt[:], st[:], Exp, scale=0.125)
                        pts[kk] = pt

                    def emit_av(kk):
                        pt = pts.pop(kk)
                        first, last = kk == 0, kk == 15
                        for u in range(2):
                            sl = slice(u * 512, (u + 1) * 512)
                            sr = slice(1024 + u * 512, 1024 + (u + 1) * 512)
                            nc.tensor.matmul(
                                av0[0:65, sl], v[:, 2 * j, kk, 0:65], pt[:, sl],
                                start=first, stop=last,
                            )
                            nc.tensor.matmul(
                                av1[0:65, sl], v[:, 2 * j + 1, kk, 0:65], pt[:, sr],
                                start=first, stop=last,
                            )

                    for kk in range(16):
                        emit_st(kk)
                        if kk > 0:
                            emit_av(kk - 1)
                    emit_av(15)

                    # early evict (frees av psum): unnormalized out^T.
                    # av1 rows 0:64 must land on partitions 64:128 -> DMA shift.
                    qsl = slice(qb * LHALF, (qb + 1) * LHALF)
                    nc.vector.tensor_copy(onT[0:64, j, qsl], av0[0:64, :])
                    tmp = wp.tile([64, LHALF], bf16, tag="tmp")
                    nc.vector.tensor_copy(tmp[:], av1[0:64, :])
                    srs = sp.tile([128, 2 * L], f32, tag="stage")
                    nc.vector.tensor_copy(srs[64:65, 0:1024], av0[64:65, :])
                    nc.vector.tensor_copy(srs[64:65, 1024:2048], av1[64:65, :])
                    # deferred normalization (overlaps the next block):
                    # srs cols 0:2048 = sums row, 2048:4096 = broadcast
                    nc.sync.dma_start(onT[64:128, j, qsl], tmp[:])
                    nc.sync.dma_start(srs[0:1, 0:2048], srs[64:65, 0:2048])
                    nc.gpsimd.partition_broadcast(
                        srs[:, 2048:4096], srs[0:1, 0:2048]
                    )
                    nc.vector.reciprocal_approx_fast(
                        srs[:, 2048:4096], srs[:, 2048:4096]
                    )
                    nc.vector.tensor_tensor(
                        onT[0:64, j, qsl], onT[0:64, j, qsl],
                        srs[0:64, 2048:3072], op=MUL,
                    )
                    nc.vector.tensor_tensor(
                        onT[64:128, j, qsl], onT[64:128, j, qsl],
                        srs[64:128, 3072:4096], op=MUL,
                    )

                def fc_chunks(qb, t8s):
                    # fc for token chunks; psum slots borrowed from av pools
                    for t8 in t8s:
                        t = qb * 8 + t8
                        pool = psav0_pool if t8 % 2 == 0 else psav1_pool
                        tag = "av0" if t8 % 2 == 0 else "av1"
                        yp = pool.tile([128, E], f32, tag=tag)
                        for c in range(4):
                            lhsT = onT[:, c, t * 128 : (t + 1) * 128]
                            for e2 in range(2):
                                nc.tensor.matmul(
                                    yp[:, e2 * 512 : (e2 + 1) * 512],
                                    lhsT,
                                    wfc[:, c, e2 * 512 : (e2 + 1) * 512],
                                    start=(c == 0),
                                    stop=(c == 3),
                                )
                        ys = yp_pool.tile([128, E], bf16, tag="ys")
                        nc.vector.tensor_tensor(ys[:], yp[:], bias[:], op=ADD)
                        nc.sync.dma_start(
                            rs_in[qb][t8 * 128 : (t8 + 1) * 128, :], ys[:]
                        )

                # qb=0 half: attention, fc, RS#1 (overlaps the qb=1 half)
                for j in range(4):
                    attn_block(0, j)
                fc_chunks(0, range(8))
                nc.gpsimd.collective_compute(
                    "ReduceScatter", ADD, replica_groups=PAIRS,
                    ins=[rs_in[0].opt()], outs=[rs_out1.opt()],
                )
                # qb=1 half; RS#1's output DMA is issued mid-way (RS#1 done)
                attn_block(1, 0)
                nc.gpsimd.dma_start(out[0 : LHALF // 2, :], rs_out1[:])
                for j in range(1, 4):
                    attn_block(1, j)
                fc_chunks(1, range(8))
                nc.gpsimd.collective_compute(
                    "ReduceScatter", ADD, replica_groups=PAIRS,
                    ins=[rs_in[1].opt()],
                    outs=[rs_out2[0].opt()],
                )

            # SWDGE cast bf16 -> f32 on the way out
            nc.gpsimd.dma_start(out[LHALF // 2 : LHALF, :], rs_out2[0][:])

    nc.finalize()
    return nc


def _prep_inputs(X, W_qkv, b_qkv, W_fc, b_fc):
    """Host-side shard + permute + cast. Returns in_maps for 8 cores."""
    X = np.asarray(X, dtype=np.float32)
    W_qkv = np.asarray(W_qkv, dtype=np.float32)
    b_qkv = np.asarray(b_qkv, dtype=np.float32)
    W_fc = np.asarray(W_fc, dtype=np.float32)
    b_fc = np.asarray(b_fc, dtype=np.float32)

    in_maps = []
    bfc_half = (0.5 * b_fc).astype(np.float32).reshape(1, E)
    for c in range(NCORES):
        b, g = divmod(c, 2)
        heads = np.arange(g * H8, (g + 1) * H8)
        # column order: all Q feats (head-major), then K, then V
        cols = np.concatenate(
            [
                np.concatenate([h * 3 * DK + off + np.arange(DK) for h in heads])
                for off in (0, DK, 2 * DK)
            ]
        )
        wq_sh = W_qkv[:, cols].astype(ml_dtypes.bfloat16)
        bq_sh = b_qkv[cols].astype(np.float32).reshape(12, 128).T.copy()
        wfc_sh = W_fc[g * FO : (g + 1) * FO, :].astype(ml_dtypes.bfloat16)
        in_maps.append(
            {
                "x": np.ascontiguousarray(X[b].T).astype(ml_dtypes.bfloat16),
                "w_qkv": wq_sh,
                "b_qkv": np.ascontiguousarray(bq_sh),
                "w_fc": wfc_sh,
                "b_fc": bfc_half,
            }
        )
    return in_maps


def run_kernel(inputs, trace=False):
    if "nc" not in _CACHE:
        _CACHE["nc"] = build_nc()
    nc = _CACHE["nc"]
    in_maps = _prep_inputs(**inputs)
    res = bass_utils.run_bass_kernel_spmd(
        nc, in_maps, core_ids=list(range(NCORES)), trace=trace
    )
    Y = np.empty((B, L, E), dtype=np.float32)
    Q2 = LHALF // 2
    for c in range(NCORES):
        b, g = divmod(c, 2)
        o = res.results[c]["out"]
        # RS#1 scattered tokens [0:1024]; RS#2 scattered [1024:2048]
        Y[b, g * Q2 : (g + 1) * Q2, :] = o[0:Q2]
        Y[b, LHALF + g * Q2 : LHALF + (g + 1) * Q2, :] = o[Q2 : 2 * Q2]
    return Y, res


def kernel(X, W_qkv, b_qkv, W_fc, b_fc):
    Y, _ = run_kernel(
        dict(X=X, W_qkv=W_qkv, b_qkv=b_qkv, W_fc=W_fc, b_fc=b_fc), trace=False
    )
    return Y



# revision 20
# speedup vs baseline: 1.1455x; 1.1455x over previous
"""Multi-head attention (B=4, L=2048, E=1024, H=16, DK=64) on 8 TRN2 cores.

Sharding: core c -> (batch b = c//2, head-group g = c%2 of 8 heads).

Single deep pipeline per core, designed around the Scalar-engine (ACT)
exp roofline (~1 elem/lane/cycle @1.2GHz -> ~252-294us for the 33.5M
softmax elements per core):
  - attention processed in 16 blocks of (head-pair j, 512-query chunk),
    16 key-chunks (kk) each; scores S^T go to a 2-tile PSUM ring
    ([128,1024] each) so exp(kk+1) never waits on PSUM reuse and the
    ACT engine runs back-to-back;
  - the qkv projection (12 feature-tiles x 4 L-chunks) and the fc
    matmuls are chopped into ~2-matmul units and pumped into the PE
    queue inside the attention kk-loop, filling the PE slack under exp;
  - AV uses the ones-column trick (K=65) for free softmax row-sums;
    normalization is deferred per block (DVE+gpsimd+DMA-shift);
  - fc output is ReduceScattered per 512-token chunk (4 small RS) so
    the collective tail is short.

PSUM budget: st ring 2x2 banks + av0/av1 1+1 + aux(qkv/fc) 2x1 = 8.

Self-contained: hardcodes all shapes; requires only the concourse stack.
"""

import numpy as np
import ml_dtypes

try:
    import axon_prof

    axon_prof.install()
except Exception:
    pass

import concourse.mybir as mybir
import concourse.tile as tile
from concourse import bacc
from concourse import bass_utils

B, L, E = 4, 2048, 1024
H, DK = 16, 64
H8 = 8                      # heads per core
F = H8 * 3 * DK             # qkv features per core = 1536
FO = H8 * DK                # attn-out features per core = 512
NCORES = 8
NLC = 4                     # L-chunks for qkv (512 tokens each)
LC = L // NLC               # 512
NQC = 4                     # query chunks (512 queries each)
QC = L // NQC               # 512
NKK = 16                    # key chunks of 128
OUTR = L // 2               # out rows per core (4 qc x 256 tokens)

f32 = mybir.dt.float32
bf16 = mybir.dt.bfloat16
Exp = mybir.ActivationFunctionType.Exp
MUL = mybir.AluOpType.mult
ADD = mybir.AluOpType.add

_CACHE = {}
DEBUG_DUMPS = False


def build_nc():
    nc = bacc.Bacc("TRN2", target_bir_lowering=False, debug=False, num_devices=NCORES)

    # x arrives already transposed (host-side); w_qkv is ft-major [12, E, 128]
    x = nc.dram_tensor("x", [E, L], bf16, kind="ExternalInput")
    w_qkv = nc.dram_tensor("w_qkv", [12, E, 128], bf16, kind="ExternalInput")
    b_qkv = nc.dram_tensor("b_qkv", [128, 12], f32, kind="ExternalInput")
    w_fc = nc.dram_tensor("w_fc", [FO, E], bf16, kind="ExternalInput")
    b_fc = nc.dram_tensor("b_fc", [1, E], f32, kind="ExternalInput")
    out = nc.dram_tensor("out", [OUTR, E], f32, kind="ExternalOutput")
    if DEBUG_DUMPS:
        dbg_qt = nc.dram_tensor("dbg_qt", [128, 4 * L], bf16, kind="ExternalOutput")
        dbg_kt = nc.dram_tensor("dbg_kt", [128, 4 * L], bf16, kind="ExternalOutput")
        dbg_v = nc.dram_tensor("dbg_v", [128, H8 * NKK * 80], bf16, kind="ExternalOutput")
        dbg_onT = nc.dram_tensor("dbg_onT", [128, 4 * L], bf16, kind="ExternalOutput")
        dbg_pt = nc.dram_tensor("dbg_pt", [128, 2 * QC], f32, kind="ExternalOutput")
        dbg_av = nc.dram_tensor("dbg_av", [128, 2 * QC], f32, kind="ExternalOutput")
        dbg_srs = nc.dram_tensor("dbg_srs", [128, 2 * QC], f32, kind="ExternalOutput")

    with tile.TileContext(nc) as tc:
        with (
            tc.tile_pool(name="persist", bufs=1) as pp,
            tc.tile_pool(name="work", bufs=3) as wp,
            tc.tile_pool(name="srsp", bufs=2) as srsp,
            tc.tile_pool(name="ys", bufs=3) as yp_pool,
            tc.tile_pool(name="dram", bufs=1, space="DRAM") as dram,
            tc.tile_pool(name="pst", bufs=2, space="PSUM") as pst_pool,
            tc.tile_pool(name="psav0", bufs=1, space="PSUM") as psav0_pool,
            tc.tile_pool(name="psav1", bufs=1, space="PSUM") as psav1_pool,
            tc.tile_pool(name="aux", bufs=2, space="PSUM") as aux_pool,
        ):
            # ---- persistent SBUF ----
            xT = pp.tile([128, 8, L], bf16, tag="xT")          # X^T  4 MiB
            wq = pp.tile([128, 12, 8, 128], bf16, tag="wq")    # 3 MiB
            bq = pp.tile([128, 12], f32, tag="bq")
            wfc = pp.tile([128, 4, E], bf16, tag="wfc")        # 1 MiB
            bias = pp.tile([128, E], f32, tag="bias")          # 0.5 MiB
            qt = pp.tile([128, 4, L], bf16, tag="qt")          # Q^T 2 MiB
            kt = pp.tile([128, 4, L], bf16, tag="kt")          # K^T 2 MiB
            vt = pp.tile([128, 4, L], bf16, tag="vt")          # V^T staging 2 MiB
            # V natural layout, 80-elem stride; col 64 holds the ones column
            # so AV matmuls with lhsT [V|1] (M=65) produce rowsums for free
            v = pp.tile([128, H8, NKK, 80], bf16, tag="v")     # 2.5 MiB
            onT = pp.tile([128, 4, L], bf16, tag="onT")        # attn out^T 2 MiB
            scratch = pp.tile([1, 4], f32, tag="scratch")
            if DEBUG_DUMPS:
                dbg_pt_s = pp.tile([128, 2 * QC], f32, tag="dbg_pt_s")
                dbg_av_s = pp.tile([128, 2 * QC], f32, tag="dbg_av_s")
                dbg_srs_s = pp.tile([128, 2 * QC], f32, tag="dbg_srs_s")

            # preload the exp table set on ACT while DMAs run
            nc.vector.memset(scratch[:], 0.0)
            nc.scalar.activation(scratch[:], scratch[:], Exp)

            # ---- input DMAs ----
            # x by L-chunk so the first qkv chunk starts after ~1MB
            for lc in range(NLC):
                for e in range(8):
                    nc.sync.dma_start(
                        xT[:, e, lc * LC : (lc + 1) * LC],
                        x[e * 128 : (e + 1) * 128, lc * LC : (lc + 1) * LC],
                    )
            nc.scalar.dma_start(bq[:], b_qkv[:])
            nc.sync.dma_start(wfc[:], w_fc.rearrange("(c p) e -> p c e", p=128))
            bfc_row = pp.tile([1, E], f32, tag="bfc_row")
            nc.sync.dma_start(bfc_row[:], b_fc[:])
            nc.gpsimd.partition_broadcast(bias[:], bfc_row[:])
            nc.vector.memset(v[:, :, :, 64:65], 1.0)

            # ---- background work stream (qkv projection, then fc) ----
            # Each unit is ~1-2 PE matmuls or one DVE evict; the attention
            # kk-loop pumps a few units per step to fill PE slack.
            bg = []

            def emit_wq_dma(j):
                def u():
                    for ft in (8 + j, 4 + j, j):
                        nc.scalar.dma_start(
                            wq[:, ft], w_qkv[ft].rearrange("(c p) m -> p c m", p=128)
                        )
                return [u]

            def qkv_chunk_units(ft, lc):
                lsl = slice(lc * LC, (lc + 1) * LC)
                state = {}

                def mk_mm(kcs):
                    def u():
                        if "ps" not in state:
                            state["ps"] = aux_pool.tile([128, LC], f32, tag="aux", name="qkvps")
                        ps = state["ps"]
                        for kc in kcs:
                            nc.tensor.matmul(
                                ps[:],
                                wq[:, ft, kc],
                                xT[:, kc, lsl],
                                start=(kc == 0),
                                stop=(kc == 7),
                            )
                    return u

                def evict():
                    ps = state.pop("ps")
                    if ft < 4:
                        dst = qt[:, ft, lsl]
                    elif ft < 8:
                        dst = kt[:, ft - 4, lsl]
                    else:
                        dst = vt[:, ft - 8, lsl]
                    nc.vector.tensor_scalar_add(dst, ps[:], bq[:, ft : ft + 1])
                    if ft >= 8:
                        j = ft - 8
                        for h in (2 * j, 2 * j + 1):
                            nc.scalar.dma_start_transpose(
                                v[:, h, lc * 4 : (lc + 1) * 4, 0:DK],
                                vt[(h % 2) * 64 : (h % 2) * 64 + 64, j, lsl],
                            )

                return [mk_mm((0, 1)), mk_mm((2, 3)), mk_mm((4, 5)), mk_mm((6, 7)), evict]

            def group_chunks(j):
                if j == 0:
                    order = [(8, 0), (4, 0), (0, 0),
                             (8, 1), (4, 1), (8, 2), (4, 2), (8, 3), (4, 3),
                             (0, 1), (0, 2), (0, 3)]
                else:
                    order = [(8 + j, lc) for lc in range(NLC)]
                    order += [(4 + j, lc) for lc in range(NLC)]
                    order += [(j, lc) for lc in range(NLC)]
                    # interleave v/k/q round-robin so kt/vt stay ahead
                    order = [order[i] for pat in range(NLC) for i in (pat, NLC + pat, 2 * NLC + pat)]
                units = []
                for ft, lc in order:
                    units.extend(qkv_chunk_units(ft, lc))
                return units

            for j in range(4):
                bg.extend(emit_wq_dma(j))
                bg.extend(group_chunks(j))

            bg_pos = [0]

            def pump(n):
                k = 0
                while k < n and bg_pos[0] < len(bg):
                    bg[bg_pos[0]]()
                    bg_pos[0] += 1
                    k += 1

            # ---- lead-in: wq dma + all of j0's K/V chunks + (0,0) ----
            # Block 0's S(kk) reads kt across the full L, so every (4,lc)
            # chunk must be EMITTED before its S — emission order defines
            # the dependency graph the tile framework can see.
            pump(46)

            # ---- attention pipeline over 16 blocks x 16 kk-steps ----
            # per step: exp(i) | S(i+1) | AV(i-1) | pump; block-end norm
            # is emitted at the following step (deferred normalization).
            PAIRS = [[0, 1], [2, 3], [4, 5], [6, 7]]
            rs_in = [
                dram.tile([QC, E], bf16, name=f"rs_in{i}", tag=f"rs_in{i}")
                for i in range(NQC)
            ]
            rs_out = [
                dram.tile([QC // 2, E], bf16, name=f"rs_out{i}", tag=f"rs_out{i}")
                for i in range(NQC)
            ]

            # block order: j-major for j=0..2, then j=3 qc-major with fc/RS
            blocks = [(j, qc) for j in range(4) for qc in range(NQC)]
            steps = [(bi, kk) for bi in range(len(blocks)) for kk in range(NKK)]

            sts = {}    # (bi, kk) -> st psum tile
            pts = {}    # (bi, kk) -> pt sbuf tile
            avs = {}    # bi -> (av0, av1)

            def emit_S(bi, kk):
                j, qc = blocks[bi]
                st = pst_pool.tile([128, 2 * QC], f32, tag="st", name="st")
                qsl = slice(qc * QC, (qc + 1) * QC)
                nc.tensor.matmul(
                    st[:, 0:QC],
                    kt[0:64, j, kk * 128 : (kk + 1) * 128],
                    qt[0:64, j, qsl],
                    start=True, stop=True,
                )
                nc.tensor.matmul(
                    st[:, QC : 2 * QC],
                    kt[64:128, j, kk * 128 : (kk + 1) * 128],
                    qt[64:128, j, qsl],
                    start=True, stop=True,
                )
                sts[(bi, kk)] = st

            def emit_exp(bi, kk):
                st = sts.pop((bi, kk))
                pt = wp.tile([128, 2 * QC], bf16, tag="pt", name="pt")
                nc.scalar.activation(pt[:], st[:], Exp, scale=0.125)
                if DEBUG_DUMPS and bi == 0 and kk == 0:
                    nc.vector.tensor_copy(dbg_pt_s[:], pt[:])
                pts[(bi, kk)] = pt

            def emit_AV(bi, kk):
                j, qc = blocks[bi]
                if kk == 0:
                    av0 = psav0_pool.tile([128, QC], f32, tag="av0", name="av0")
                    av1 = psav1_pool.tile([128, QC], f32, tag="av1", name="av1")
                    avs[bi] = (av0, av1)
                av0, av1 = avs[bi]
                pt = pts.pop((bi, kk))
                first, last = kk == 0, kk == NKK - 1
                nc.tensor.matmul(
                    av0[0:65, :], v[:, 2 * j, kk, 0:65], pt[:, 0:QC],
                    start=first, stop=last,
                )
                nc.tensor.matmul(
                    av1[0:65, :], v[:, 2 * j + 1, kk, 0:65], pt[:, QC : 2 * QC],
                    start=first, stop=last,
                )

            def emit_norm(bi):
                j, qc = blocks[bi]
                av0, av1 = avs.pop(bi)
                qsl = slice(qc * QC, (qc + 1) * QC)
                if DEBUG_DUMPS and bi == 0:
                    nc.vector.tensor_copy(dbg_av_s[:, 0:QC], av0[:])
                    nc.vector.tensor_copy(dbg_av_s[:, QC : 2 * QC], av1[:])
                # evict unnormalized out^T; av1 rows must shift to 64:128
                nc.vector.tensor_copy(onT[0:64, j, qsl], av0[0:64, :])
                tmp = wp.tile([64, QC], bf16, tag="tmp", name="tmp")
                nc.vector.tensor_copy(tmp[:], av1[0:64, :])
                srs = srsp.tile([128, 4 * QC], f32, tag="srs", name="srs")
                nc.vector.tensor_copy(srs[64:65, 0:QC], av0[64:65, :])
                nc.vector.tensor_copy(srs[64:65, QC : 2 * QC], av1[64:65, :])
                nc.sync.dma_start(onT[64:128, j, qsl], tmp[:])
                nc.sync.dma_start(srs[0:1, 0 : 2 * QC], srs[64:65, 0 : 2 * QC])
                nc.gpsimd.partition_broadcast(
                    srs[:, 2 * QC : 4 * QC], srs[0:1, 0 : 2 * QC]
                )
                nc.vector.reciprocal_approx_fast(
                    srs[:, 2 * QC : 4 * QC], srs[:, 2 * QC : 4 * QC]
                )
                if DEBUG_DUMPS and bi == 0:
                    nc.vector.tensor_copy(dbg_srs_s[:], srs[:, 2 * QC : 4 * QC])
                nc.vector.tensor_tensor(
                    onT[0:64, j, qsl], onT[0:64, j, qsl],
                    srs[0:64, 2 * QC : 3 * QC], op=MUL,
                )
                nc.vector.tensor_tensor(
                    onT[64:128, j, qsl], onT[64:128, j, qsl],
                    srs[64:128, 3 * QC : 4 * QC], op=MUL,
                )

            def fc_units(qc):
                units = []
                for t8 in range(4):
                    t = qc * 4 + t8
                    for e2 in range(2):
                        def mk(t=t, t8=t8, e2=e2):
                            st_ = {}

                            def mms():
                                yp = aux_pool.tile([128, LC], f32, tag="aux", name="fcps")
                                st_["yp"] = yp
                                for c in range(4):
                                    nc.tensor.matmul(
                                        yp[:],
                                        onT[:, c, t * 128 : (t + 1) * 128],
                                        wfc[:, c, e2 * 512 : (e2 + 1) * 512],
                                        start=(c == 0),
                                        stop=(c == 3),
                                    )

                            def evict():
                                yp = st_.pop("yp")
                                ys = yp_pool.tile([128, 512], bf16, tag="ys", name="ys")
                                nc.vector.tensor_tensor(
                                    ys[:], yp[:], bias[:, e2 * 512 : (e2 + 1) * 512],
                                    op=ADD,
                                )
                                nc.sync.dma_start(
                                    rs_in[qc][
                                        t8 * 128 : (t8 + 1) * 128,
                                        e2 * 512 : (e2 + 1) * 512,
                                    ],
                                    ys[:],
                                )

                            return [mms, evict]
                        units.extend(mk())

                def rs():
                    nc.gpsimd.collective_compute(
                        "ReduceScatter", ADD, replica_groups=PAIRS,
                        ins=[rs_in[qc].opt()], outs=[rs_out[qc].opt()],
                    )

                def odma_prev():
                    # out-DMA for the PREVIOUS qc: its RS is long done, so
                    # this never blocks the in-order gpsimd queue (norm
                    # broadcasts) behind a still-running collective.
                    p = qc - 1
                    nc.gpsimd.dma_start(
                        out[p * (QC // 2) : (p + 1) * (QC // 2), :], rs_out[p][:]
                    )

                units.append(rs)
                if qc > 0:
                    units.append(odma_prev)
                return units

            emit_S(*steps[0])
            for i, (bi, kk) in enumerate(steps):
                emit_exp(bi, kk)
                if i + 1 < len(steps):
                    emit_S(*steps[i + 1])
                if i > 0:
                    pbi, pkk = steps[i - 1]
                    emit_AV(pbi, pkk)
                    if pkk == NKK - 1:
                        emit_norm(pbi)
                        j, qc = blocks[pbi]
                        if j == 3:
                            bg.extend(fc_units(qc))
                pump(2)
            emit_AV(*steps[-1])
            emit_norm(len(blocks) - 1)
            bg.extend(fc_units(NQC - 1))
            pump(len(bg))
            # final out-DMA (for the last qc's ReduceScatter)
            nc.gpsimd.dma_start(
                out[(NQC - 1) * (QC // 2) : NQC * (QC // 2), :], rs_out[NQC - 1][:]
            )
            if DEBUG_DUMPS:
                nc.sync.dma_start(dbg_qt[:], qt[:])
                nc.sync.dma_start(dbg_kt[:], kt[:])
                nc.sync.dma_start(dbg_v[:], v[:])
                nc.sync.dma_start(dbg_onT[:], onT[:])
                nc.sync.dma_start(dbg_pt[:], dbg_pt_s[:])
                nc.sync.dma_start(dbg_av[:], dbg_av_s[:])
                nc.sync.dma_start(dbg_srs[:], dbg_srs_s[:])

    nc.finalize()
    return nc


def _prep_inputs(X, W_qkv, b_qkv, W_fc, b_fc):
    """Host-side shard + permute + cast. Returns in_maps for 8 cores."""
    X = np.asarray(X, dtype=np.float32)
    W_qkv = np.asarray(W_qkv, dtype=np.float32)
    b_qkv = np.asarray(b_qkv, dtype=np.float32)
    W_fc = np.asarray(W_fc, dtype=np.float32)
    b_fc = np.asarray(b_fc, dtype=np.float32)

    in_maps = []
    bfc_half = (0.5 * b_fc).astype(np.float32).reshape(1, E)
    for c in range(NCORES):
        b, g = divmod(c, 2)
        heads = np.arange(g * H8, (g + 1) * H8)
        # column order: all Q feats (head-major), then K, then V
        cols = np.concatenate(
            [
                np.concatenate([h * 3 * DK + off + np.arange(DK) for h in heads])
                for off in (0, DK, 2 * DK)
            ]
        )
        wq_sh = W_qkv[:, cols].astype(ml_dtypes.bfloat16)
        # ft-major [12, E, 128]
        wq_ft = np.ascontiguousarray(
            wq_sh.reshape(E, 12, 128).transpose(1, 0, 2)
        )
        bq_sh = b_qkv[cols].astype(np.float32).reshape(12, 128).T.copy()
        wfc_sh = W_fc[g * FO : (g + 1) * FO, :].astype(ml_dtypes.bfloat16)
        in_maps.append(
            {
                "x": np.ascontiguousarray(X[b].T).astype(ml_dtypes.bfloat16),
                "w_qkv": wq_ft,
                "b_qkv": np.ascontiguousarray(bq_sh),
                "w_fc": wfc_sh,
                "b_fc": bfc_half,
            }
        )
    return in_maps


def run_kernel(inputs, trace=False):
    if "nc" not in _CACHE:
        _CACHE["nc"] = build_nc()
    nc = _CACHE["nc"]
    in_maps = _prep_inputs(**inputs)
    res = bass_utils.run_bass_kernel_spmd(
        nc, in_maps, core_ids=list(range(NCORES)), trace=trace
    )
    Y = np.empty((B, L, E), dtype=np.float32)
    Q2 = QC // 2
    for c in range(NCORES):
        b, g = divmod(c, 2)
        o = res.results[c]["out"]
        for qc in range(NQC):
            Y[b, qc * QC + g * Q2 : qc * QC + (g + 1) * Q2, :] = o[
                qc * Q2 : (qc + 1) * Q2
            ]
    return Y, res


def kernel(X, W_qkv, b_qkv, W_fc, b_fc):
    Y, _ = run_kernel(
        dict(X=X, W_qkv=W_qkv, b_qkv=b_qkv, W_fc=W_fc, b_fc=b_fc), trace=False
    )
    return Y


# revision 28
# speedup vs baseline: 1.2383x; 1.0811x over previous
"""Multi-head attention (B=4, L=2048, E=1024, H=16, DK=64) on 8 TRN2 cores.

Sharding: core c -> (batch b = c//2, head-group g = c%2 of 8 heads).

Single deep pipeline per core, designed around the Scalar-engine (ACT)
exp roofline (~1 elem/lane/cycle @1.2GHz -> ~252-294us for the 33.5M
softmax elements per core):
  - attention processed in 16 blocks of (head-pair j, 512-query chunk),
    16 key-chunks (kk) each; scores S^T go to a 2-tile PSUM ring
    ([128,1024] each) so exp(kk+1) never waits on PSUM reuse and the
    ACT engine runs back-to-back;
  - the qkv projection (12 feature-tiles x 4 L-chunks) and the fc
    matmuls are chopped into ~2-matmul units and pumped into the PE
    queue inside the attention kk-loop, filling the PE slack under exp;
  - AV uses the ones-column trick (K=65) for free softmax row-sums;
    normalization is deferred per block (DVE+gpsimd+DMA-shift);
  - fc output is ReduceScattered per 512-token chunk (4 small RS) so
    the collective tail is short.

PSUM budget: st ring 2x2 banks + av0/av1 1+1 + aux(qkv/fc) 2x1 = 8.

Self-contained: hardcodes all shapes; requires only the concourse stack.
"""

import numpy as np
import ml_dtypes

try:
    import axon_prof

    axon_prof.install()
except Exception:
    pass

import concourse.mybir as mybir
import concourse.tile as tile
from concourse import bacc
from concourse import bass_utils

B, L, E = 4, 2048, 1024
H, DK = 16, 64
H8 = 8                      # heads per core
F = H8 * 3 * DK             # qkv features per core = 1536
FO = H8 * DK                # attn-out features per core = 512
NCORES = 8
NLC = 4                     # L-chunks for qkv (512 tokens each)
LC = L // NLC               # 512
NQC = 4                     # query chunks (512 queries each)
QC = L // NQC               # 512
NKK = 16                    # key chunks of 128
OUTR = L // 2               # out rows per core (4 qc x 256 tokens)

f32 = mybir.dt.float32
bf16 = mybir.dt.bfloat16
Exp = mybir.ActivationFunctionType.Exp
MUL = mybir.AluOpType.mult
ADD = mybir.AluOpType.add

_CACHE = {}
DEBUG_DUMPS = False


def build_nc():
    nc = bacc.Bacc("TRN2", target_bir_lowering=False, debug=False, num_devices=NCORES)

    # x arrives already transposed (host-side); w_qkv is [ft, p, kc, m] so
    # each DMA row is 2KB contiguous (good descriptor size)
    x = nc.dram_tensor("x", [E, L], bf16, kind="ExternalInput")
    w_qkv = nc.dram_tensor("w_qkv", [12, 128, 8 * 128], bf16, kind="ExternalInput")
    b_qkv = nc.dram_tensor("b_qkv", [128, 12], f32, kind="ExternalInput")
    w_fc = nc.dram_tensor("w_fc", [FO, E], bf16, kind="ExternalInput")
    b_fc = nc.dram_tensor("b_fc", [1, E], f32, kind="ExternalInput")
    out = nc.dram_tensor("out", [OUTR, E], f32, kind="ExternalOutput")
    if DEBUG_DUMPS:
        dbg_qt = nc.dram_tensor("dbg_qt", [128, 4 * L], bf16, kind="ExternalOutput")
        dbg_kt = nc.dram_tensor("dbg_kt", [128, 4 * L], bf16, kind="ExternalOutput")
        dbg_v = nc.dram_tensor("dbg_v", [128, H8 * NKK * 80], bf16, kind="ExternalOutput")
        dbg_onT = nc.dram_tensor("dbg_onT", [128, 4 * L], bf16, kind="ExternalOutput")
        dbg_pt = nc.dram_tensor("dbg_pt", [128, 2 * QC], f32, kind="ExternalOutput")
        dbg_av = nc.dram_tensor("dbg_av", [128, 2 * QC], f32, kind="ExternalOutput")
        dbg_srs = nc.dram_tensor("dbg_srs", [128, 2 * QC], f32, kind="ExternalOutput")

    with tile.TileContext(nc) as tc:
        with (
            tc.tile_pool(name="persist", bufs=1) as pp,
            tc.tile_pool(name="work", bufs=3) as wp,
            tc.tile_pool(name="srsp", bufs=2) as srsp,
            tc.tile_pool(name="ys", bufs=3) as yp_pool,
            tc.tile_pool(name="dram", bufs=1, space="DRAM") as dram,
            tc.tile_pool(name="pst", bufs=2, space="PSUM") as pst_pool,
            tc.tile_pool(name="psav0", bufs=1, space="PSUM") as psav0_pool,
            tc.tile_pool(name="psav1", bufs=1, space="PSUM") as psav1_pool,
            tc.tile_pool(name="aux", bufs=2, space="PSUM") as aux_pool,
        ):
            # ---- persistent SBUF ----
            xT = pp.tile([128, 8, L], bf16, tag="xT")          # X^T  4 MiB
            wq = pp.tile([128, 12, 8, 128], bf16, tag="wq")    # 3 MiB
            bq = pp.tile([128, 12], f32, tag="bq")
            wfc = pp.tile([128, 4, E], bf16, tag="wfc")        # 1 MiB
            bias = pp.tile([128, E], f32, tag="bias")          # 0.5 MiB
            qt = pp.tile([128, 4, L], bf16, tag="qt")          # Q^T 2 MiB
            kt = pp.tile([128, 4, L], bf16, tag="kt")          # K^T 2 MiB
            vt = pp.tile([128, 4, L], bf16, tag="vt")          # V^T staging 2 MiB
            # V natural layout, 80-elem stride; col 64 holds the ones column
            # so AV matmuls with lhsT [V|1] (M=65) produce rowsums for free
            v = pp.tile([128, H8, NKK, 80], bf16, tag="v")     # 2.5 MiB
            onT = pp.tile([128, 4, L], bf16, tag="onT")        # attn out^T 2 MiB
            scratch = pp.tile([1, 4], f32, tag="scratch")
            if DEBUG_DUMPS:
                dbg_pt_s = pp.tile([128, 2 * QC], f32, tag="dbg_pt_s")
                dbg_av_s = pp.tile([128, 2 * QC], f32, tag="dbg_av_s")
                dbg_srs_s = pp.tile([128, 2 * QC], f32, tag="dbg_srs_s")

            # preload the exp table set on ACT while DMAs run
            nc.vector.memset(scratch[:], 0.0)
            nc.scalar.activation(scratch[:], scratch[:], Exp)

            # ---- input DMAs ----
            # inputs (x, wq, biases) go on the scalar queue; the sync queue
            # carries intra-kernel DMAs (v transposes, norm shifts, fc out)
            # so they are not stuck behind 7MB of input traffic.
            nc.scalar.dma_start(bq[:], b_qkv[:])
            for ft in (8, 4, 0):
                nc.scalar.dma_start(wq[:, ft], w_qkv[ft].rearrange("p (c m) -> p c m", c=8))
            # x by L-chunk so the first qkv chunk starts after ~1MB
            for lc in range(NLC):
                for e in range(8):
                    nc.scalar.dma_start(
                        xT[:, e, lc * LC : (lc + 1) * LC],
                        x[e * 128 : (e + 1) * 128, lc * LC : (lc + 1) * LC],
                    )
            nc.sync.dma_start(wfc[:], w_fc.rearrange("(c p) e -> p c e", p=128))
            bfc_row = pp.tile([1, E], f32, tag="bfc_row")
            nc.sync.dma_start(bfc_row[:], b_fc[:])
            nc.gpsimd.partition_broadcast(bias[:], bfc_row[:])
            nc.vector.memset(v[:, :, :, 64:65], 1.0)

            # ---- background work stream (qkv projection, then fc) ----
            # Each unit is ~1-2 PE matmuls or one DVE evict; the attention
            # kk-loop pumps a few units per step to fill PE slack.
            bg = []

            def emit_wq_dma(j):
                def u():
                    if j == 0:
                        return  # j0's wq DMAs were issued upfront
                    for ft in (8 + j, 4 + j, j):
                        nc.scalar.dma_start(
                            wq[:, ft], w_qkv[ft].rearrange("p (c m) -> p c m", c=8)
                        )
                return [u]

            def qkv_chunk_units(ft, lc):
                lsl = slice(lc * LC, (lc + 1) * LC)
                state = {}

                def mk_mm(kcs):
                    def u():
                        if "ps" not in state:
                            state["ps"] = aux_pool.tile([128, LC], f32, tag="aux", name="qkvps")
                        ps = state["ps"]
                        for kc in kcs:
                            nc.tensor.matmul(
                                ps[:],
                                wq[:, ft, kc],
                                xT[:, kc, lsl],
                                start=(kc == 0),
                                stop=(kc == 7),
                            )
                    return u

                def evict():
                    ps = state.pop("ps")
                    if ft < 4:
                        dst = qt[:, ft, lsl]
                    elif ft < 8:
                        dst = kt[:, ft - 4, lsl]
                    else:
                        dst = vt[:, ft - 8, lsl]
                    nc.vector.tensor_scalar_add(dst, ps[:], bq[:, ft : ft + 1])
                    if ft >= 8:
                        j = ft - 8
                        for h in (2 * j, 2 * j + 1):
                            nc.sync.dma_start_transpose(
                                v[:, h, lc * 4 : (lc + 1) * 4, 0:DK],
                                vt[(h % 2) * 64 : (h % 2) * 64 + 64, j, lsl],
                            )

                return [mk_mm((0, 1)), mk_mm((2, 3)), mk_mm((4, 5)), mk_mm((6, 7)), evict]

            def group_chunks(j):
                if j == 0:
                    order = [(8, 0), (4, 0), (0, 0),
                             (8, 1), (4, 1), (8, 2), (4, 2), (8, 3), (4, 3),
                             (0, 1), (0, 2), (0, 3)]
                else:
                    order = [(8 + j, lc) for lc in range(NLC)]
                    order += [(4 + j, lc) for lc in range(NLC)]
                    order += [(j, lc) for lc in range(NLC)]
                    # interleave v/k/q round-robin so kt/vt stay ahead
                    order = [order[i] for pat in range(NLC) for i in (pat, NLC + pat, 2 * NLC + pat)]
                units = []
                for ft, lc in order:
                    units.extend(qkv_chunk_units(ft, lc))
                return units

            for j in range(4):
                bg.extend(emit_wq_dma(j))
                bg.extend(group_chunks(j))

            bg_pos = [0]

            def pump(n):
                k = 0
                while k < n and bg_pos[0] < len(bg):
                    bg[bg_pos[0]]()
                    bg_pos[0] += 1
                    k += 1

            # ---- lead-in: wq dma + all of j0's K/V chunks + (0,0) ----
            # Block 0's S(kk) reads kt across the full L, so every (4,lc)
            # chunk must be EMITTED before its S — emission order defines
            # the dependency graph the tile framework can see.
            pump(46)

            # ---- attention pipeline over 16 blocks x 16 kk-steps ----
            # per step: exp(i) | S(i+1) | AV(i-1) | pump; block-end norm
            # is emitted at the following step (deferred normalization).
            PAIRS = [[0, 1], [2, 3], [4, 5], [6, 7]]
            rs_in = [
                dram.tile([QC, E], bf16, name=f"rs_in{i}", tag=f"rs_in{i}")
                for i in range(NQC)
            ]
            rs_out = [
                dram.tile([QC // 2, E], bf16, name=f"rs_out{i}", tag=f"rs_out{i}")
                for i in range(NQC)
            ]

            # anti-diagonal block order: early blocks favor low j (whose
            # qkv is ready first) while each qc's 4 head-pairs complete
            # progressively, so fc+RS for qc0..2 run mid-kernel and only
            # qc3's fc/RS remains in the tail.
            blocks = [
                (j, d - j)
                for d in range(7)
                for j in range(4)
                if 0 <= d - j <= 3
            ]
            steps = [(bi, kk) for bi in range(len(blocks)) for kk in range(NKK)]

            sts = {}    # (bi, kk) -> st psum tile
            pts = {}    # (bi, kk) -> pt sbuf tile
            avs = {}    # bi -> (av0, av1)

            def emit_S(bi, kk):
                j, qc = blocks[bi]
                st = pst_pool.tile([128, 2 * QC], f32, tag="st", name="st")
                qsl = slice(qc * QC, (qc + 1) * QC)
                nc.tensor.matmul(
                    st[:, 0:QC],
                    kt[0:64, j, kk * 128 : (kk + 1) * 128],
                    qt[0:64, j, qsl],
                    start=True, stop=True,
                )
                nc.tensor.matmul(
                    st[:, QC : 2 * QC],
                    kt[64:128, j, kk * 128 : (kk + 1) * 128],
                    qt[64:128, j, qsl],
                    start=True, stop=True,
                )
                sts[(bi, kk)] = st

            def emit_exp(bi, kk):
                st = sts.pop((bi, kk))
                pt = wp.tile([128, 2 * QC], bf16, tag="pt", name="pt")
                nc.scalar.activation(pt[:], st[:], Exp, scale=0.125)
                if DEBUG_DUMPS and bi == 0 and kk == 0:
                    nc.vector.tensor_copy(dbg_pt_s[:], pt[:])
                pts[(bi, kk)] = pt

            def emit_AV(bi, kk):
                j, qc = blocks[bi]
                if kk == 0:
                    av0 = psav0_pool.tile([128, QC], f32, tag="av0", name="av0")
                    av1 = psav1_pool.tile([128, QC], f32, tag="av1", name="av1")
                    avs[bi] = (av0, av1)
                av0, av1 = avs[bi]
                pt = pts.pop((bi, kk))
                first, last = kk == 0, kk == NKK - 1
                nc.tensor.matmul(
                    av0[0:65, :], v[:, 2 * j, kk, 0:65], pt[:, 0:QC],
                    start=first, stop=last,
                )
                nc.tensor.matmul(
                    av1[0:65, :], v[:, 2 * j + 1, kk, 0:65], pt[:, QC : 2 * QC],
                    start=first, stop=last,
                )

            def emit_norm(bi):
                j, qc = blocks[bi]
                av0, av1 = avs.pop(bi)
                qsl = slice(qc * QC, (qc + 1) * QC)
                if DEBUG_DUMPS and bi == 0:
                    nc.vector.tensor_copy(dbg_av_s[:, 0:QC], av0[:])
                    nc.vector.tensor_copy(dbg_av_s[:, QC : 2 * QC], av1[:])
                # evict unnormalized out^T; av1 rows must shift to 64:128
                nc.vector.tensor_copy(onT[0:64, j, qsl], av0[0:64, :])
                tmp = wp.tile([64, QC], bf16, tag="tmp", name="tmp")
                nc.vector.tensor_copy(tmp[:], av1[0:64, :])
                srs = srsp.tile([128, 4 * QC], f32, tag="srs", name="srs")
                nc.vector.tensor_copy(srs[64:65, 0:QC], av0[64:65, :])
                nc.vector.tensor_copy(srs[64:65, QC : 2 * QC], av1[64:65, :])
                nc.sync.dma_start(onT[64:128, j, qsl], tmp[:])
                nc.sync.dma_start(srs[0:1, 0 : 2 * QC], srs[64:65, 0 : 2 * QC])
                nc.gpsimd.partition_broadcast(
                    srs[:, 2 * QC : 4 * QC], srs[0:1, 0 : 2 * QC]
                )
                nc.vector.reciprocal_approx_fast(
                    srs[:, 2 * QC : 4 * QC], srs[:, 2 * QC : 4 * QC]
                )
                if DEBUG_DUMPS and bi == 0:
                    nc.vector.tensor_copy(dbg_srs_s[:], srs[:, 2 * QC : 4 * QC])
                nc.vector.tensor_tensor(
                    onT[0:64, j, qsl], onT[0:64, j, qsl],
                    srs[0:64, 2 * QC : 3 * QC], op=MUL,
                )
                nc.vector.tensor_tensor(
                    onT[64:128, j, qsl], onT[64:128, j, qsl],
                    srs[64:128, 3 * QC : 4 * QC], op=MUL,
                )

            def fc_units(qc):
                units = []
                for t8 in range(4):
                    t = qc * 4 + t8
                    for e2 in range(2):
                        def mk(t=t, t8=t8, e2=e2):
                            st_ = {}

                            def mk_mms(cs):
                                def mms():
                                    if "yp" not in st_:
                                        st_["yp"] = aux_pool.tile(
                                            [128, LC], f32, tag="aux", name="fcps"
                                        )
                                    yp = st_["yp"]
                                    for c in cs:
                                        nc.tensor.matmul(
                                            yp[:],
                                            onT[:, c, t * 128 : (t + 1) * 128],
                                            wfc[:, c, e2 * 512 : (e2 + 1) * 512],
                                            start=(c == 0),
                                            stop=(c == 3),
                                        )
                                return mms

                            def evict():
                                yp = st_.pop("yp")
                                ys = yp_pool.tile([128, 512], bf16, tag="ys", name="ys")
                                nc.vector.tensor_tensor(
                                    ys[:], yp[:], bias[:, e2 * 512 : (e2 + 1) * 512],
                                    op=ADD,
                                )
                                nc.sync.dma_start(
                                    rs_in[qc][
                                        t8 * 128 : (t8 + 1) * 128,
                                        e2 * 512 : (e2 + 1) * 512,
                                    ],
                                    ys[:],
                                )

                            return [mk_mms((0, 1)), mk_mms((2, 3)), evict]
                        units.extend(mk())

                def rs():
                    nc.gpsimd.collective_compute(
                        "ReduceScatter", ADD, replica_groups=PAIRS,
                        ins=[rs_in[qc].opt()], outs=[rs_out[qc].opt()],
                    )

                def odma_prev():
                    # out-DMA for the PREVIOUS qc: its RS is long done, so
                    # this never blocks the in-order gpsimd queue (norm
                    # broadcasts) behind a still-running collective.
                    p = qc - 1
                    nc.gpsimd.dma_start(
                        out[p * (QC // 2) : (p + 1) * (QC // 2), :], rs_out[p][:]
                    )

                units.append(rs)
                if qc > 0:
                    units.append(odma_prev)
                return units

            emit_S(*steps[0])
            for i, (bi, kk) in enumerate(steps):
                emit_exp(bi, kk)
                if i + 1 < len(steps):
                    emit_S(*steps[i + 1])
                if i > 0:
                    pbi, pkk = steps[i - 1]
                    emit_AV(pbi, pkk)
                    if pkk == NKK - 1:
                        emit_norm(pbi)
                        j, qc = blocks[pbi]
                        if j == 3:
                            bg.extend(fc_units(qc))
                pump(2)
            emit_AV(*steps[-1])
            emit_norm(len(blocks) - 1)
            bg.extend(fc_units(NQC - 1))
            pump(len(bg))
            # final out-DMA (for the last qc's ReduceScatter)
            nc.gpsimd.dma_start(
                out[(NQC - 1) * (QC // 2) : NQC * (QC // 2), :], rs_out[NQC - 1][:]
            )
            if DEBUG_DUMPS:
                nc.sync.dma_start(dbg_qt[:], qt[:])
                nc.sync.dma_start(dbg_kt[:], kt[:])
                nc.sync.dma_start(dbg_v[:], v[:])
                nc.sync.dma_start(dbg_onT[:], onT[:])
                nc.sync.dma_start(dbg_pt[:], dbg_pt_s[:])
                nc.sync.dma_start(dbg_av[:], dbg_av_s[:])
                nc.sync.dma_start(dbg_srs[:], dbg_srs_s[:])

    nc.finalize()
    return nc


def _prep_inputs(X, W_qkv, b_qkv, W_fc, b_fc):
    """Host-side shard + permute + cast. Returns in_maps for 8 cores."""
    X = np.asarray(X, dtype=np.float32)
    W_qkv = np.asarray(W_qkv, dtype=np.float32)
    b_qkv = np.asarray(b_qkv, dtype=np.float32)
    W_fc = np.asarray(W_fc, dtype=np.float32)
    b_fc = np.asarray(b_fc, dtype=np.float32)

    in_maps = []
    bfc_half = (0.5 * b_fc).astype(np.float32).reshape(1, E)
    for c in range(NCORES):
        b, g = divmod(c, 2)
        heads = np.arange(g * H8, (g + 1) * H8)
        # column order: all Q feats (head-major), then K, then V
        cols = np.concatenate(
            [
                np.concatenate([h * 3 * DK + off + np.arange(DK) for h in heads])
                for off in (0, DK, 2 * DK)
            ]
        )
        wq_sh = W_qkv[:, cols].astype(ml_dtypes.bfloat16)
        # [ft, p, kc*m] so each SBUF partition row is one contiguous 2KB DMA
        wq_ft = np.ascontiguousarray(
            wq_sh.reshape(8, 128, 12, 128).transpose(2, 1, 0, 3).reshape(12, 128, 8 * 128)
        )
        bq_sh = b_qkv[cols].astype(np.float32).reshape(12, 128).T.copy()
        wfc_sh = W_fc[g * FO : (g + 1) * FO, :].astype(ml_dtypes.bfloat16)
        in_maps.append(
            {
                "x": np.ascontiguousarray(X[b].T).astype(ml_dtypes.bfloat16),
                "w_qkv": wq_ft,
                "b_qkv": np.ascontiguousarray(bq_sh),
                "w_fc": wfc_sh,
                "b_fc": bfc_half,
            }
        )
    return in_maps


def run_kernel(inputs, trace=False):
    if "nc" not in _CACHE:
        _CACHE["nc"] = build_nc()
    nc = _CACHE["nc"]
    in_maps = _prep_inputs(**inputs)
    res = bass_utils.run_bass_kernel_spmd(
        nc, in_maps, core_ids=list(range(NCORES)), trace=trace
    )
    Y = np.empty((B, L, E), dtype=np.float32)
    Q2 = QC // 2
    for c in range(NCORES):
        b, g = divmod(c, 2)
        o = res.results[c]["out"]
        for qc in range(NQC):
            Y[b, qc * QC + g * Q2 : qc * QC + (g + 1) * Q2, :] = o[
                qc * Q2 : (qc + 1) * Q2
            ]
    return Y, res


def kernel(X, W_qkv, b_qkv, W_fc, b_fc):
    Y, _ = run_kernel(
        dict(X=X, W_qkv=W_qkv, b_qkv=b_qkv, W_fc=W_fc, b_fc=b_fc), trace=False
    )
    return Y


# revision 31
# speedup vs baseline: 1.2540x; 1.0126x over previous
"""Multi-head attention (B=4, L=2048, E=1024, H=16, DK=64) on 8 TRN2 cores.

Sharding: core c -> (batch b = c//2, head-group g = c%2 of 8 heads).

Single deep pipeline per core, designed around the Scalar-engine (ACT)
exp roofline (~1 elem/lane/cycle @1.2GHz -> ~252-294us for the 33.5M
softmax elements per core):
  - attention processed in 16 blocks of (head-pair j, 512-query chunk),
    16 key-chunks (kk) each; scores S^T go to a 2-tile PSUM ring
    ([128,1024] each) so exp(kk+1) never waits on PSUM reuse and the
    ACT engine runs back-to-back;
  - the qkv projection (12 feature-tiles x 4 L-chunks) and the fc
    matmuls are chopped into ~2-matmul units and pumped into the PE
    queue inside the attention kk-loop, filling the PE slack under exp;
  - AV uses the ones-column trick (K=65) for free softmax row-sums;
    normalization is deferred per block (DVE+gpsimd+DMA-shift);
  - fc output is ReduceScattered per 512-token chunk (4 small RS) so
    the collective tail is short.

PSUM budget: st ring 2x2 banks + av0/av1 1+1 + aux(qkv/fc) 2x1 = 8.

Self-contained: hardcodes all shapes; requires only the concourse stack.
"""

import numpy as np
import ml_dtypes

try:
    import axon_prof

    axon_prof.install()
except Exception:
    pass

import concourse.mybir as mybir
import concourse.tile as tile
from concourse import bacc
from concourse import bass_utils

B, L, E = 4, 2048, 1024
H, DK = 16, 64
H8 = 8                      # heads per core
F = H8 * 3 * DK             # qkv features per core = 1536
FO = H8 * DK                # attn-out features per core = 512
NCORES = 8
NLC = 4                     # L-chunks for qkv (512 tokens each)
LC = L // NLC               # 512
NQC = 4                     # query chunks (512 queries each)
QC = L // NQC               # 512
NKK = 16                    # key chunks of 128
OUTR = L // 2               # out rows per core (4 qc x 256 tokens)

f32 = mybir.dt.float32
bf16 = mybir.dt.bfloat16
Exp = mybir.ActivationFunctionType.Exp
MUL = mybir.AluOpType.mult
ADD = mybir.AluOpType.add

_CACHE = {}
DEBUG_DUMPS = False


def build_nc():
    nc = bacc.Bacc("TRN2", target_bir_lowering=False, debug=False, num_devices=NCORES)

    # x arrives already transposed (host-side); w_qkv is [ft, p, kc, m] so
    # each DMA row is 2KB contiguous (good descriptor size)
    x = nc.dram_tensor("x", [E, L], bf16, kind="ExternalInput")
    w_qkv = nc.dram_tensor("w_qkv", [12, 128, 8 * 128], bf16, kind="ExternalInput")
    b_qkv = nc.dram_tensor("b_qkv", [128, 12], f32, kind="ExternalInput")
    w_fc = nc.dram_tensor("w_fc", [FO, E], bf16, kind="ExternalInput")
    b_fc = nc.dram_tensor("b_fc", [1, E], f32, kind="ExternalInput")
    out = nc.dram_tensor("out", [OUTR, E], f32, kind="ExternalOutput")
    if DEBUG_DUMPS:
        dbg_qt = nc.dram_tensor("dbg_qt", [128, 4 * L], bf16, kind="ExternalOutput")
        dbg_kt = nc.dram_tensor("dbg_kt", [128, 4 * L], bf16, kind="ExternalOutput")
        dbg_v = nc.dram_tensor("dbg_v", [128, H8 * NKK * 80], bf16, kind="ExternalOutput")
        dbg_onT = nc.dram_tensor("dbg_onT", [128, 4 * L], bf16, kind="ExternalOutput")
        dbg_pt = nc.dram_tensor("dbg_pt", [128, 2 * QC], f32, kind="ExternalOutput")
        dbg_av = nc.dram_tensor("dbg_av", [128, 2 * QC], f32, kind="ExternalOutput")
        dbg_srs = nc.dram_tensor("dbg_srs", [128, 2 * QC], f32, kind="ExternalOutput")

    with tile.TileContext(nc) as tc:
        with (
            tc.tile_pool(name="persist", bufs=1) as pp,
            tc.tile_pool(name="work", bufs=3) as wp,
            tc.tile_pool(name="srsp", bufs=2) as srsp,
            tc.tile_pool(name="ys", bufs=3) as yp_pool,
            tc.tile_pool(name="dram", bufs=1, space="DRAM") as dram,
            tc.tile_pool(name="pst", bufs=2, space="PSUM") as pst_pool,
            tc.tile_pool(name="psav0", bufs=1, space="PSUM") as psav0_pool,
            tc.tile_pool(name="psav1", bufs=1, space="PSUM") as psav1_pool,
            tc.tile_pool(name="aux", bufs=2, space="PSUM") as aux_pool,
        ):
            # ---- persistent SBUF ----
            xT = pp.tile([128, 8, L], bf16, tag="xT")          # X^T  4 MiB
            wq = pp.tile([128, 12, 8, 128], bf16, tag="wq")    # 3 MiB
            bq = pp.tile([128, 12], f32, tag="bq")
            wfc = pp.tile([128, 4, E], bf16, tag="wfc")        # 1 MiB
            bias = pp.tile([128, E], f32, tag="bias")          # 0.5 MiB
            qt = pp.tile([128, 4, L], bf16, tag="qt")          # Q^T 2 MiB
            kt = pp.tile([128, 4, L], bf16, tag="kt")          # K^T 2 MiB
            vt = pp.tile([128, 4, L], bf16, tag="vt")          # V^T staging 2 MiB
            # V natural layout, 80-elem stride; col 64 holds the ones column
            # so AV matmuls with lhsT [V|1] (M=65) produce rowsums for free
            v = pp.tile([128, H8, NKK, 80], bf16, tag="v")     # 2.5 MiB
            onT = pp.tile([128, 4, L], bf16, tag="onT")        # attn out^T 2 MiB
            scratch = pp.tile([1, 4], f32, tag="scratch")
            if DEBUG_DUMPS:
                dbg_pt_s = pp.tile([128, 2 * QC], f32, tag="dbg_pt_s")
                dbg_av_s = pp.tile([128, 2 * QC], f32, tag="dbg_av_s")
                dbg_srs_s = pp.tile([128, 2 * QC], f32, tag="dbg_srs_s")

            # preload the exp table set on ACT while DMAs run
            nc.vector.memset(scratch[:], 0.0)
            nc.scalar.activation(scratch[:], scratch[:], Exp)

            # ---- input DMAs ----
            # inputs (x, wq, biases) go on the scalar queue; the sync queue
            # carries intra-kernel DMAs (v transposes, norm shifts, fc out)
            # so they are not stuck behind 7MB of input traffic.
            nc.scalar.dma_start(bq[:], b_qkv[:])
            for ft in (8, 4, 0):
                nc.scalar.dma_start(wq[:, ft], w_qkv[ft].rearrange("p (c m) -> p c m", c=8))
            # x by L-chunk, split across both HWDGE queues for bandwidth
            for lc in range(NLC):
                for e in range(8):
                    eng = nc.scalar if e % 2 == 0 else nc.sync
                    eng.dma_start(
                        xT[:, e, lc * LC : (lc + 1) * LC],
                        x[e * 128 : (e + 1) * 128, lc * LC : (lc + 1) * LC],
                    )
            nc.sync.dma_start(wfc[:], w_fc.rearrange("(c p) e -> p c e", p=128))
            bfc_row = pp.tile([1, E], f32, tag="bfc_row")
            nc.sync.dma_start(bfc_row[:], b_fc[:])
            nc.gpsimd.partition_broadcast(bias[:], bfc_row[:])
            nc.vector.memset(v[:, :, :, 64:65], 1.0)

            # ---- background work stream (qkv projection, then fc) ----
            # Each unit is ~1-2 PE matmuls or one DVE evict; the attention
            # kk-loop pumps a few units per step to fill PE slack.
            bg = []

            def emit_wq_dma(j):
                def u():
                    if j == 0:
                        return  # j0's wq DMAs were issued upfront
                    for ft in (8 + j, 4 + j, j):
                        nc.scalar.dma_start(
                            wq[:, ft], w_qkv[ft].rearrange("p (c m) -> p c m", c=8)
                        )
                return [u]

            def qkv_chunk_units(ft, lc):
                lsl = slice(lc * LC, (lc + 1) * LC)
                state = {}

                def mk_mm(kcs):
                    def u():
                        if "ps" not in state:
                            state["ps"] = aux_pool.tile([128, LC], f32, tag="aux", name="qkvps")
                        ps = state["ps"]
                        for kc in kcs:
                            nc.tensor.matmul(
                                ps[:],
                                wq[:, ft, kc],
                                xT[:, kc, lsl],
                                start=(kc == 0),
                                stop=(kc == 7),
                            )
                    return u

                def evict():
                    ps = state.pop("ps")
                    if ft < 4:
                        dst = qt[:, ft, lsl]
                    elif ft < 8:
                        dst = kt[:, ft - 4, lsl]
                    else:
                        dst = vt[:, ft - 8, lsl]
                    nc.vector.tensor_scalar_add(dst, ps[:], bq[:, ft : ft + 1])
                    if ft >= 8:
                        j = ft - 8
                        for h in (2 * j, 2 * j + 1):
                            nc.sync.dma_start_transpose(
                                v[:, h, lc * 4 : (lc + 1) * 4, 0:DK],
                                vt[(h % 2) * 64 : (h % 2) * 64 + 64, j, lsl],
                            )

                return [mk_mm((0, 1)), mk_mm((2, 3)), mk_mm((4, 5)), mk_mm((6, 7)), evict]

            def group_chunks(j):
                if j == 0:
                    order = [(8, 0), (4, 0), (0, 0),
                             (8, 1), (4, 1), (8, 2), (4, 2), (8, 3), (4, 3),
                             (0, 1), (0, 2), (0, 3)]
                else:
                    order = [(8 + j, lc) for lc in range(NLC)]
                    order += [(4 + j, lc) for lc in range(NLC)]
                    order += [(j, lc) for lc in range(NLC)]
                    # interleave v/k/q round-robin so kt/vt stay ahead
                    order = [order[i] for pat in range(NLC) for i in (pat, NLC + pat, 2 * NLC + pat)]
                units = []
                for ft, lc in order:
                    units.extend(qkv_chunk_units(ft, lc))
                return units

            for j in range(4):
                bg.extend(emit_wq_dma(j))
                bg.extend(group_chunks(j))

            bg_pos = [0]

            def pump(n):
                k = 0
                while k < n and bg_pos[0] < len(bg):
                    bg[bg_pos[0]]()
                    bg_pos[0] += 1
                    k += 1

            # ---- lead-in: wq dma + chunks (8,0),(4,0),(0,0) ----
            # Attention starts after just 3 chunks; the early pump rate (5
            # units/step) keeps every (4,lc)/(8,lc) chunk EMITTED before the
            # S/AV that reads it (S(kk) at step kk-1 needs chunk (4,kk//4):
            # evict unit index 5*(kk//4)+25 <= 16+5*(kk-2) for kk>=4).
            pump(16)

            # ---- attention pipeline over 16 blocks x 16 kk-steps ----
            # per step: exp(i) | S(i+1) | AV(i-1) | pump; block-end norm
            # is emitted at the following step (deferred normalization).
            PAIRS = [[0, 1], [2, 3], [4, 5], [6, 7]]
            rs_in = [
                dram.tile([QC, E], bf16, name=f"rs_in{i}", tag=f"rs_in{i}")
                for i in range(NQC)
            ]
            rs_out = [
                dram.tile([QC // 2, E], bf16, name=f"rs_out{i}", tag=f"rs_out{i}")
                for i in range(NQC)
            ]

            # anti-diagonal block order: early blocks favor low j (whose
            # qkv is ready first) while each qc's 4 head-pairs complete
            # progressively, so fc+RS for qc0..2 run mid-kernel and only
            # qc3's fc/RS remains in the tail.
            blocks = [
                (j, d - j)
                for d in range(7)
                for j in range(4)
                if 0 <= d - j <= 3
            ]
            steps = [(bi, kk) for bi in range(len(blocks)) for kk in range(NKK)]

            sts = {}    # (bi, kk) -> st psum tile
            pts = {}    # (bi, kk) -> pt sbuf tile
            avs = {}    # bi -> (av0, av1)

            def emit_S(bi, kk):
                j, qc = blocks[bi]
                st = pst_pool.tile([128, 2 * QC], f32, tag="st", name="st")
                qsl = slice(qc * QC, (qc + 1) * QC)
                nc.tensor.matmul(
                    st[:, 0:QC],
                    kt[0:64, j, kk * 128 : (kk + 1) * 128],
                    qt[0:64, j, qsl],
                    start=True, stop=True,
                )
                nc.tensor.matmul(
                    st[:, QC : 2 * QC],
                    kt[64:128, j, kk * 128 : (kk + 1) * 128],
                    qt[64:128, j, qsl],
                    start=True, stop=True,
                )
                sts[(bi, kk)] = st

            def emit_exp(bi, kk):
                st = sts.pop((bi, kk))
                pt = wp.tile([128, 2 * QC], bf16, tag="pt", name="pt")
                nc.scalar.activation(pt[:], st[:], Exp, scale=0.125)
                if DEBUG_DUMPS and bi == 0 and kk == 0:
                    nc.vector.tensor_copy(dbg_pt_s[:], pt[:])
                pts[(bi, kk)] = pt

            def emit_AV(bi, kk):
                j, qc = blocks[bi]
                if kk == 0:
                    av0 = psav0_pool.tile([128, QC], f32, tag="av0", name="av0")
                    av1 = psav1_pool.tile([128, QC], f32, tag="av1", name="av1")
                    avs[bi] = (av0, av1)
                av0, av1 = avs[bi]
                pt = pts.pop((bi, kk))
                first, last = kk == 0, kk == NKK - 1
                nc.tensor.matmul(
                    av0[0:65, :], v[:, 2 * j, kk, 0:65], pt[:, 0:QC],
                    start=first, stop=last,
                )
                nc.tensor.matmul(
                    av1[0:65, :], v[:, 2 * j + 1, kk, 0:65], pt[:, QC : 2 * QC],
                    start=first, stop=last,
                )

            def emit_norm(bi):
                j, qc = blocks[bi]
                av0, av1 = avs.pop(bi)
                qsl = slice(qc * QC, (qc + 1) * QC)
                if DEBUG_DUMPS and bi == 0:
                    nc.vector.tensor_copy(dbg_av_s[:, 0:QC], av0[:])
                    nc.vector.tensor_copy(dbg_av_s[:, QC : 2 * QC], av1[:])
                # evict unnormalized out^T; av1 rows must shift to 64:128
                nc.vector.tensor_copy(onT[0:64, j, qsl], av0[0:64, :])
                tmp = wp.tile([64, QC], bf16, tag="tmp", name="tmp")
                nc.vector.tensor_copy(tmp[:], av1[0:64, :])
                srs = srsp.tile([128, 4 * QC], f32, tag="srs", name="srs")
                nc.vector.tensor_copy(srs[64:65, 0:QC], av0[64:65, :])
                nc.vector.tensor_copy(srs[64:65, QC : 2 * QC], av1[64:65, :])
                nc.sync.dma_start(onT[64:128, j, qsl], tmp[:])
                nc.sync.dma_start(srs[0:1, 0 : 2 * QC], srs[64:65, 0 : 2 * QC])
                nc.gpsimd.partition_broadcast(
                    srs[:, 2 * QC : 4 * QC], srs[0:1, 0 : 2 * QC]
                )
                nc.vector.reciprocal_approx_fast(
                    srs[:, 2 * QC : 4 * QC], srs[:, 2 * QC : 4 * QC]
                )
                if DEBUG_DUMPS and bi == 0:
                    nc.vector.tensor_copy(dbg_srs_s[:], srs[:, 2 * QC : 4 * QC])
                nc.vector.tensor_tensor(
                    onT[0:64, j, qsl], onT[0:64, j, qsl],
                    srs[0:64, 2 * QC : 3 * QC], op=MUL,
                )
                nc.vector.tensor_tensor(
                    onT[64:128, j, qsl], onT[64:128, j, qsl],
                    srs[64:128, 3 * QC : 4 * QC], op=MUL,
                )

            def fc_units(qc):
                units = []
                for t8 in range(4):
                    t = qc * 4 + t8
                    for e2 in range(2):
                        def mk(t=t, t8=t8, e2=e2):
                            st_ = {}

                            def mk_mms(cs):
                                def mms():
                                    if "yp" not in st_:
                                        st_["yp"] = aux_pool.tile(
                                            [128, LC], f32, tag="aux", name="fcps"
                                        )
                                    yp = st_["yp"]
                                    for c in cs:
                                        nc.tensor.matmul(
                                            yp[:],
                                            onT[:, c, t * 128 : (t + 1) * 128],
                                            wfc[:, c, e2 * 512 : (e2 + 1) * 512],
                                            start=(c == 0),
                                            stop=(c == 3),
                                        )
                                return mms

                            def evict():
                                yp = st_.pop("yp")
                                ys = yp_pool.tile([128, 512], bf16, tag="ys", name="ys")
                                nc.vector.tensor_tensor(
                                    ys[:], yp[:], bias[:, e2 * 512 : (e2 + 1) * 512],
                                    op=ADD,
                                )
                                nc.sync.dma_start(
                                    rs_in[qc][
                                        t8 * 128 : (t8 + 1) * 128,
                                        e2 * 512 : (e2 + 1) * 512,
                                    ],
                                    ys[:],
                                )

                            return [mk_mms((0, 1)), mk_mms((2, 3)), evict]
                        units.extend(mk())

                def rs():
                    nc.gpsimd.collective_compute(
                        "ReduceScatter", ADD, replica_groups=PAIRS,
                        ins=[rs_in[qc].opt()], outs=[rs_out[qc].opt()],
                    )

                def odma_prev():
                    # out-DMA for the PREVIOUS qc: its RS is long done, so
                    # this never blocks the in-order gpsimd queue (norm
                    # broadcasts) behind a still-running collective.
                    p = qc - 1
                    nc.gpsimd.dma_start(
                        out[p * (QC // 2) : (p + 1) * (QC // 2), :], rs_out[p][:]
                    )

                units.append(rs)
                if qc > 0:
                    units.append(odma_prev)
                return units

            emit_S(*steps[0])
            for i, (bi, kk) in enumerate(steps):
                emit_exp(bi, kk)
                if i + 1 < len(steps):
                    emit_S(*steps[i + 1])
                if i > 0:
                    pbi, pkk = steps[i - 1]
                    emit_AV(pbi, pkk)
                    if pkk == NKK - 1:
                        emit_norm(pbi)
                        j, qc = blocks[pbi]
                        if j == 3:
                            bg.extend(fc_units(qc))
                pump(5 if i < 16 else 2)
            emit_AV(*steps[-1])
            emit_norm(len(blocks) - 1)
            bg.extend(fc_units(NQC - 1))
            pump(len(bg))
            # final out-DMA (for the last qc's ReduceScatter)
            nc.gpsimd.dma_start(
                out[(NQC - 1) * (QC // 2) : NQC * (QC // 2), :], rs_out[NQC - 1][:]
            )
            if DEBUG_DUMPS:
                nc.sync.dma_start(dbg_qt[:], qt[:])
                nc.sync.dma_start(dbg_kt[:], kt[:])
                nc.sync.dma_start(dbg_v[:], v[:])
                nc.sync.dma_start(dbg_onT[:], onT[:])
                nc.sync.dma_start(dbg_pt[:], dbg_pt_s[:])
                nc.sync.dma_start(dbg_av[:], dbg_av_s[:])
                nc.sync.dma_start(dbg_srs[:], dbg_srs_s[:])

    nc.finalize()
    return nc


def _prep_inputs(X, W_qkv, b_qkv, W_fc, b_fc):
    """Host-side shard + permute + cast. Returns in_maps for 8 cores."""
    X = np.asarray(X, dtype=np.float32)
    W_qkv = np.asarray(W_qkv, dtype=np.float32)
    b_qkv = np.asarray(b_qkv, dtype=np.float32)
    W_fc = np.asarray(W_fc, dtype=np.float32)
    b_fc = np.asarray(b_fc, dtype=np.float32)

    in_maps = []
    bfc_half = (0.5 * b_fc).astype(np.float32).reshape(1, E)
    for c in range(NCORES):
        b, g = divmod(c, 2)
        heads = np.arange(g * H8, (g + 1) * H8)
        # column order: all Q feats (head-major), then K, then V
        cols = np.concatenate(
            [
                np.concatenate([h * 3 * DK + off + np.arange(DK) for h in heads])
                for off in (0, DK, 2 * DK)
            ]
        )
        wq_sh = W_qkv[:, cols].astype(ml_dtypes.bfloat16)
        # [ft, p, kc*m] so each SBUF partition row is one contiguous 2KB DMA
        wq_ft = np.ascontiguousarray(
            wq_sh.reshape(8, 128, 12, 128).transpose(2, 1, 0, 3).reshape(12, 128, 8 * 128)
        )
        bq_sh = b_qkv[cols].astype(np.float32).reshape(12, 128).T.copy()
        wfc_sh = W_fc[g * FO : (g + 1) * FO, :].astype(ml_dtypes.bfloat16)
        in_maps.append(
            {
                "x": np.ascontiguousarray(X[b].T).astype(ml_dtypes.bfloat16),
                "w_qkv": wq_ft,
                "b_qkv": np.ascontiguousarray(bq_sh),
                "w_fc": wfc_sh,
                "b_fc": bfc_half,
            }
        )
    return in_maps


def run_kernel(inputs, trace=False):
    if "nc" not in _CACHE:
        _CACHE["nc"] = build_nc()
    nc = _CACHE["nc"]
    in_maps = _prep_inputs(**inputs)
    res = bass_utils.run_bass_kernel_spmd(
        nc, in_maps, core_ids=list(range(NCORES)), trace=trace
    )
    Y = np.empty((B, L, E), dtype=np.float32)
    Q2 = QC // 2
    for c in range(NCORES):
        b, g = divmod(c, 2)
        o = res.results[c]["out"]
        for qc in range(NQC):
            Y[b, qc * QC + g * Q2 : qc * QC + (g + 1) * Q2, :] = o[
                qc * Q2 : (qc + 1) * Q2
            ]
    return Y, res


def kernel(X, W_qkv, b_qkv, W_fc, b_fc):
    Y, _ = run_kernel(
        dict(X=X, W_qkv=W_qkv, b_qkv=b_qkv, W_fc=W_fc, b_fc=b_fc), trace=False
    )
    return Y
